# revision 28
# baseline (speedup 1.0000x reference)
"""Trainium2 Bass kernel for the Tacotron2-style decoder (nn_Decoder_70377334112524).

Strategy: pure data parallel over batch (B=32 -> 4 per core x 8 cores).
Per core, the computation is restructured into 4 phases:
  0: prenet over all frames + G1a = prenet_out @ Wa_ih[:, :256].T  (batched matmuls)
  1: sequential scan t=0..T-1 of attention-LSTM + location-sensitive attention
  2: G2 = [ah|ctx] @ Wd_ih.T batched; pre_mel/pre_gate = ctx @ proj/gate ctx-part
  3: sequential scan of decoder-LSTM + projections (only W_hh recurrent matmul)

Weights live in SBUF (bf16); all accumulation fp32 in PSUM; softmax/state fp32.
Dropout masks are data-independent: reproduced exactly on host with JAX threefry
(CPU backend), matching the reference's per-step bernoulli draws inside scan.
"""

import numpy as np
import ml_dtypes

import concourse.bass as bass
import concourse.tile as tile
from concourse import bacc, mybir
from concourse.bass import ds
from concourse.bass_utils import run_bass_kernel_spmd
from concourse.masks import make_identity

AF = mybir.ActivationFunctionType
F32 = mybir.dt.float32
BF16 = mybir.dt.bfloat16

B, T_ENC, T_DEC = 32, 768, 400
N_MELS, PRENET, ENC, RNN, ATT, LOC_F, LOC_K = 80, 256, 512, 1024, 128, 32, 31
NCORES = 8
BS = B // NCORES  # 4
GD = 4 * RNN      # 4096
bf = ml_dtypes.bfloat16

# gate-row permutation: per 512-quad [i|f|o|g] so the cell updates chunkwise
_PERM = np.concatenate([np.concatenate(
    [np.arange(0, 512) + qd * 512,          # i
     np.arange(1024, 1536) + qd * 512,      # f
     np.arange(3072, 3584) + qd * 512,      # o
     np.arange(2048, 2560) + qd * 512])     # g
    for qd in (0, 1)])


# ---------------------------------------------------------------- host helpers

def _masks(td):
    """Exact dropout masks as the reference draws them (threefry, CPU)."""
    import jax
    import jax.numpy as jnp
    cpu = jax.devices("cpu")[0]
    with jax.default_device(cpu):
        key = jax.random.key(42)
        k1, k2, k3 = jax.random.split(key, 3)
        m1 = jax.random.bernoulli(k1, 0.5, (T_DEC + 1, B, PRENET))
        m2 = jax.random.bernoulli(k2, 0.5, (T_DEC + 1, B, PRENET))
        sk = jax.random.split(k3, T_DEC * 2).reshape(T_DEC, 2)

        def mstep(c, ks):
            return c, (jax.random.bernoulli(ks[0], 0.9, (B, RNN)),
                       jax.random.bernoulli(ks[1], 0.9, (B, RNN)))
        _, (mah, mdh) = jax.lax.scan(mstep, 0, sk)
    m1 = np.asarray(m1[:td]).astype(np.float32) * 2.0
    m2 = np.asarray(m2[:td]).astype(np.float32) * 2.0
    mah = np.asarray(mah[:td]).astype(np.float32) / 0.9
    mdh = np.asarray(mdh[:td]).astype(np.float32) / 0.9
    return m1, m2, mah, mdh


def _ktiles(wT, kt):
    """(K, N) -> (128, kt, N) with [p, i, n] = wT[i*128+p, n]."""
    K, N = wT.shape
    assert K == kt * 128
    return np.ascontiguousarray(np.transpose(wT.reshape(kt, 128, N), (1, 0, 2)))


def host_prep(memory, decoder_inputs, memory_lengths, params, td=T_DEC):
    p = {k: np.asarray(v, np.float32) for k, v in params.items()}
    memory = np.asarray(memory, np.float32)
    decoder_inputs = np.asarray(decoder_inputs, np.float32)
    memory_lengths = np.asarray(memory_lengths)

    for bn in ("pre_b1", "pre_b2", "arnn_b_ih", "arnn_b_hh", "drnn_b_ih",
               "drnn_b_hh", "proj_b", "gate_b"):
        assert np.abs(p[bn]).max() == 0.0, f"nonzero bias {bn} not supported"

    m1, m2, mah, mdh = _masks(td)

    # shared (replicated) weight tensors
    wa = np.concatenate([p["arnn_w_ih"][:, PRENET:], p["arnn_w_hh"]], axis=1)[_PERM]
    shared = {
        "w1T": p["pre_w1"].T.astype(bf),                              # (80,256)
        "w2T": _ktiles(p["pre_w2"].T, 2).astype(bf),                  # (128,2,256)
        "wpT": _ktiles(p["arnn_w_ih"][:, :PRENET][_PERM].T, 2).astype(bf),  # (128,2,4096)
        "wa": _ktiles(wa.T, 12).astype(bf),                           # (128,12,4096)
        "wq": _ktiles(p["attn_q_w"].T, 8).astype(bf),                 # (128,8,128)
        "wflat": np.transpose(p["loc_conv_w"], (1, 2, 0)).reshape(62, LOC_F).astype(bf),
        "ldT": p["loc_dense_w"].T.astype(bf),                         # (32,128)
        "vT": p["attn_v"].T.astype(bf),                               # (128,1)
        "wd": _ktiles(p["drnn_w_hh"][_PERM].T, 8).astype(bf),         # (128,8,4096)
        "wdi": _ktiles(p["drnn_w_ih"][_PERM].T, 12).astype(bf),       # (128,12,4096)
        "projdh": _ktiles(p["proj_w"][:, :RNN].T, 8).astype(bf),      # (128,8,80)
        "projctx": _ktiles(p["proj_w"][:, RNN:].T, 4).astype(bf),     # (128,4,80)
        "gatedh": _ktiles(p["gate_w"][:, :RNN].T, 8).astype(bf),      # (128,8,1)
        "gatectx": _ktiles(p["gate_w"][:, RNN:].T, 4).astype(bf),     # (128,4,1)
    }

    pm = np.einsum("btd,fd->btf", memory, p["attn_mem_w"])            # (B,TE,128)
    x = np.concatenate([np.zeros((1, B, N_MELS), np.float32),
                        np.transpose(decoder_inputs, (2, 0, 1))], 0)[:td]
    maskadd = np.where(np.arange(T_ENC)[None, :] >= memory_lengths[:, None],
                       np.float32(-1e30), np.float32(0.0)).astype(np.float32)

    in_maps = []
    for c in range(NCORES):
        sl = slice(c * BS, (c + 1) * BS)
        mem_c = memory[sl]                                            # (4,TE,512)
        d = dict(shared)
        d["xT"] = np.ascontiguousarray(np.transpose(x[:, sl, :], (2, 0, 1))).astype(bf)  # (80,td,4)
        d["m1T"] = np.ascontiguousarray(
            np.transpose(m1[:, sl, :].reshape(td, BS, 2, 128), (3, 2, 0, 1))).astype(bf)  # (128,2,td,4)
        d["m2T"] = np.ascontiguousarray(
            np.transpose(m2[:, sl, :].reshape(td, BS, 2, 128), (3, 2, 0, 1))).astype(bf)
        d["mah"] = mah[:, sl, :].astype(bf)                           # (td,4,1024)
        d["mdh"] = mdh[:, sl, :].astype(bf)
        d["pm"] = np.ascontiguousarray(np.transpose(pm[sl], (2, 0, 1))).astype(bf)  # (128,4,TE)
        d["memT2"] = np.ascontiguousarray(
            np.transpose(mem_c.reshape(BS, 6, 128, ENC), (2, 1, 0, 3))).astype(bf)  # (128,6,4,512)
        d["maskadd"] = np.ascontiguousarray(maskadd[sl])              # (4,TE)
        in_maps.append(d)
    return in_maps


# ---------------------------------------------------------------- bass builder

def build_nc(td=T_DEC):
    nc = bacc.Bacc(None, target_bir_lowering=False)

    def DI(name, shape, dt):
        return nc.dram_tensor(name, list(shape), dt, kind="ExternalInput")

    # inputs
    w1T = DI("w1T", (80, 256), BF16)
    w2T = DI("w2T", (128, 2, 256), BF16)
    wpT = DI("wpT", (128, 2, GD), BF16)
    wa = DI("wa", (128, 12, GD), BF16)
    wq = DI("wq", (128, 8, 128), BF16)
    wflat = DI("wflat", (62, LOC_F), BF16)
    ldT = DI("ldT", (LOC_F, 128), BF16)
    vT = DI("vT", (128, 1), BF16)
    wd = DI("wd", (128, 8, GD), BF16)
    wdi = DI("wdi", (128, 12, GD), BF16)
    projdh = DI("projdh", (128, 8, N_MELS), BF16)
    projctx = DI("projctx", (128, 4, N_MELS), BF16)
    gatedh = DI("gatedh", (128, 8, 1), BF16)
    gatectx = DI("gatectx", (128, 4, 1), BF16)
    xT = DI("xT", (80, td, BS), BF16)
    m1T = DI("m1T", (128, 2, td, BS), BF16)
    m2T = DI("m2T", (128, 2, td, BS), BF16)
    mah_i = DI("mah", (td, BS, RNN), BF16)
    mdh_i = DI("mdh", (td, BS, RNN), BF16)
    pm_i = DI("pm", (128, BS, T_ENC), BF16)
    memT2_i = DI("memT2", (128, 6, BS, ENC), BF16)
    maskadd_i = DI("maskadd", (BS, T_ENC), F32)

    # outputs
    mel_o = nc.dram_tensor("mel_o", [td, BS, N_MELS], F32, kind="ExternalOutput")
    gate_o = nc.dram_tensor("gate_o", [td, BS], F32, kind="ExternalOutput")
    align_o = nc.dram_tensor("align_o", [td, BS, T_ENC], F32, kind="ExternalOutput")

    NT = (td * BS + 127) // 128  # n-tiles over (t,b) rows

    with tile.TileContext(nc) as tc:
        with tc.tile_pool(name="dram", bufs=1, space="DRAM") as dpool:
            g1a_d = dpool.tile([td, BS, GD], BF16)
            awpad_d = dpool.tile([2 * BS, T_ENC + 2 * (LOC_K // 2)], BF16)
            ah_d = dpool.tile([td, BS, RNN], BF16)
            ctx_d = dpool.tile([td, BS, ENC], BF16)
            g2_d = dpool.tile([td, BS, GD], BF16)
            premel_d = dpool.tile([td, BS, N_MELS], F32)
            pregate_d = dpool.tile([td, BS, 1], F32)

            # ============================================ phase 0: prenet + G1a
            with (
                tc.tile_pool(name="p0c", bufs=1) as p0c,
                tc.tile_pool(name="p0w", bufs=3) as p0w,
                tc.tile_pool(name="p0ps", bufs=4, space="PSUM") as p0ps,
            ):
                w1s = p0c.tile([80, 256], BF16)
                w2s = p0c.tile([128, 2, 256], BF16)
                wps = p0c.tile([128, 2, GD], BF16)
                xTs = p0c.tile([80, td * BS], BF16)
                m1s = p0c.tile([128, 2, td * BS], BF16)
                m2s = p0c.tile([128, 2, td * BS], BF16)
                h1 = p0c.tile([128, 2, td * BS], BF16)
                h2 = p0c.tile([128, 2, td * BS], BF16)
                nc.gpsimd.dma_start(out=w1s, in_=w1T[:, :])
                nc.gpsimd.dma_start(out=w2s, in_=w2T[:, :, :])
                nc.gpsimd.dma_start(out=wps, in_=wpT[:, :, :])
                nc.gpsimd.dma_start(out=xTs, in_=xT.rearrange("c t b -> c (t b)"))
                nc.gpsimd.dma_start(out=m1s, in_=m1T.rearrange("p c t b -> p c (t b)"))
                nc.gpsimd.dma_start(out=m2s, in_=m2T.rearrange("p c t b -> p c (t b)"))

                NROW = td * BS
                nch = (NROW + 511) // 512
                for mc in range(2):
                    for nn in range(nch):
                        nsl = ds(nn * 512, min(512, NROW - nn * 512))
                        ps = p0ps.tile([128, 512], F32)
                        nc.tensor.matmul(ps[:, 0:nsl.size], w1s[:, ds(mc * 128, 128)],
                                         xTs[:, nsl], start=True, stop=True)
                        nc.scalar.activation(out=h1[:, mc, nsl], in_=ps[:, 0:nsl.size],
                                             func=AF.Relu)
                        nc.vector.tensor_mul(h1[:, mc, nsl], h1[:, mc, nsl], m1s[:, mc, nsl])
                for mc in range(2):
                    for nn in range(nch):
                        nsl = ds(nn * 512, min(512, NROW - nn * 512))
                        ps = p0ps.tile([128, 512], F32)
                        for kt in range(2):
                            nc.tensor.matmul(ps[:, 0:nsl.size], w2s[:, kt, ds(mc * 128, 128)],
                                             h1[:, kt, nsl], start=(kt == 0), stop=(kt == 1))
                        nc.scalar.activation(out=h2[:, mc, nsl], in_=ps[:, 0:nsl.size],
                                             func=AF.Relu)
                        nc.vector.tensor_mul(h2[:, mc, nsl], h2[:, mc, nsl], m2s[:, mc, nsl])
                # G1a = pn @ Wp.T  -> (td,4,4096) bf16
                for nt in range(NT):
                    nrows = min(128, NROW - nt * 128)
                    for gc in range(8):
                        ps = p0ps.tile([128, 512], F32)
                        for kt in range(2):
                            nc.tensor.matmul(ps[:nrows, :], h2[:, kt, ds(nt * 128, nrows)],
                                             wps[:, kt, ds(gc * 512, 512)],
                                             start=(kt == 0), stop=(kt == 1))
                        ob = p0w.tile([128, 512], BF16)
                        nc.scalar.activation(out=ob[:nrows, :], in_=ps[:nrows, :], func=AF.Copy)
                        nc.gpsimd.dma_start(
                            out=g1a_d.rearrange("t b g -> (t b) g")[ds(nt * 128, nrows),
                                                                    ds(gc * 512, 512)],
                            in_=ob[:nrows, :])

            # ============================================ phase 1: attention scan
            with (
                tc.tile_pool(name="c1", bufs=1) as c1,
                tc.tile_pool(name="st", bufs=1) as st,
                tc.tile_pool(name="wk", bufs=1) as wk,
                tc.tile_pool(name="ps1", bufs=4, space="PSUM") as ps1,
                tc.tile_pool(name="psE", bufs=1, space="PSUM") as psE,
            ):
                wa_s = c1.tile([128, 12, GD], BF16)
                memT2_s = c1.tile([128, 6, BS * ENC], BF16)
                pm_s = c1.tile([128, BS * T_ENC], BF16)
                wq_s = c1.tile([128, 8, 128], BF16)
                wflat_s = c1.tile([62, LOC_F], BF16)
                ldT_s = c1.tile([LOC_F, 128], BF16)
                v_s = c1.tile([128, 1], BF16)
                maskadd_s = c1.tile([BS, T_ENC], F32)
                ident = c1.tile([128, 128], F32)
                ident_bf = c1.tile([128, 128], BF16)
                nc.gpsimd.dma_start(out=wa_s, in_=wa[:, :, :])
                nc.gpsimd.dma_start(out=memT2_s, in_=memT2_i.rearrange("p k b d -> p k (b d)"))
                nc.gpsimd.dma_start(out=pm_s, in_=pm_i.rearrange("p b t -> p (b t)"))
                nc.gpsimd.dma_start(out=wq_s, in_=wq[:, :, :])
                nc.gpsimd.dma_start(out=wflat_s, in_=wflat[:, :])
                nc.gpsimd.dma_start(out=ldT_s, in_=ldT[:, :])
                nc.gpsimd.dma_start(out=v_s, in_=vT[:, :])
                nc.gpsimd.dma_start(out=maskadd_s, in_=maskadd_i[:, :])
                make_identity(nc, ident)
                nc.vector.tensor_copy(ident_bf, ident)

                # persistent state
                actT = st.tile([128, 12, BS], BF16)   # [ctx(4) | ah(8)] feature-T
                c_sb = st.tile([BS, RNN], F32)
                awc_sb = st.tile([BS, T_ENC], F32)
                zpad = st.tile([2 * BS, T_ENC + 30], BF16)
                nc.vector.memset(actT, 0.0)
                nc.vector.memset(c_sb, 0.0)
                nc.vector.memset(awc_sb, 0.0)
                nc.vector.memset(zpad, 0.0)
                nc.gpsimd.dma_start(out=awpad_d, in_=zpad)

                PADW = T_ENC + 30

                with tc.For_i(0, td) as iv:
                    g1a_t = wk.tile([BS, GD], BF16, tag="g1a")
                    nc.gpsimd.dma_start(out=g1a_t, in_=g1a_d[ds(iv, 1), :, :])
                    mah_t = wk.tile([BS, RNN], BF16, tag="mah")
                    nc.gpsimd.dma_start(out=mah_t, in_=mah_i[ds(iv, 1), :, :])
                    xt = wk.tile([62, BS, T_ENC], BF16, tag="xt")
                    for i2 in range(2):
                        nc.gpsimd.dma_start(
                            out=xt[ds(i2 * 31, 31), :, :],
                            in_=bass.AP(tensor=awpad_d.tensor,
                                        offset=awpad_d.offset + i2 * BS * PADW,
                                        ap=[[1, 31], [PADW, BS], [1, T_ENC]]))

                    # ---- attention LSTM gates: per 512-quad [i|f|o|g] chunks
                    ah_b = wk.tile([BS, RNN], F32, tag="ahb")
                    for qd in range(2):
                        sg = {}
                        for ti, tn in enumerate("ifog"):
                            ps = ps1.tile([BS, 512], F32, tag="ps_s")
                            gc = qd * 4 + ti
                            nc.tensor.matmul(ps, ident_bf[0:BS, 0:BS],
                                             g1a_t[:, ds(gc * 512, 512)], start=True, stop=False)
                            for kt in range(12):
                                nc.tensor.matmul(ps, actT[:, kt, :],
                                                 wa_s[:, kt, ds(gc * 512, 512)],
                                                 start=False, stop=(kt == 11))
                            sg[tn] = wk.tile([BS, 512], F32, tag=f"sg{tn}", name=f"sg{tn}")
                            nc.scalar.activation(out=sg[tn], in_=ps,
                                                 func=(AF.Tanh if tn == "g" else AF.Sigmoid))
                        qsl = ds(qd * 512, 512)
                        t1 = wk.tile([BS, 512], F32, tag="t1")
                        t2 = wk.tile([BS, 512], F32, tag="t2")
                        nc.vector.tensor_mul(t1, sg["f"], c_sb[:, qsl])
                        nc.vector.tensor_mul(t2, sg["i"], sg["g"])
                        nc.vector.tensor_add(c_sb[:, qsl], t1, t2)
                        tnc = wk.tile([BS, 512], F32, tag="t1", name="tnc")
                        nc.scalar.activation(out=tnc, in_=c_sb[:, qsl], func=AF.Tanh)
                        nc.vector.tensor_mul(sg["o"], sg["o"], mah_t[:, qsl])
                        nc.vector.tensor_mul(ah_b[:, qsl], sg["o"], tnc)
                    ah_bf = wk.tile([BS, RNN], BF16, tag="ahbf")
                    nc.vector.tensor_copy(ah_bf, ah_b)
                    nc.gpsimd.dma_start(out=ah_d[ds(iv, 1), :, :], in_=ah_bf)
                    # transposes ah -> actT[:, 4+j, :]
                    for j in range(8):
                        pst = ps1.tile([128, BS], F32, tag="ps_s")
                        nc.tensor.transpose(pst, ah_b[:, ds(j * 128, 128)], ident[0:BS, 0:BS])
                        nc.vector.tensor_copy(actT[:, 4 + j, :], pst)

                    # ---- q = ah @ Wq.T  (feature-T out)
                    psq = ps1.tile([128, BS], F32, tag="ps_s")
                    for j in range(8):
                        nc.tensor.matmul(psq, wq_s[:, j, :], actT[:, 4 + j, :],
                                         start=(j == 0), stop=(j == 7))
                    q_sb = wk.tile([128, BS], F32, tag="qsb")
                    nc.scalar.activation(out=q_sb, in_=psq, func=AF.Copy)

                    # ---- conv (as matmul from shifted windows) -> convout (32,(b,t))
                    convout = wk.tile([LOC_F, BS * T_ENC], BF16, tag="conv")
                    xt_f = xt.rearrange("p b t -> p (b t)")
                    for cc in range(6):
                        psc = ps1.tile([LOC_F, 512], F32, tag="ps_s")
                        nc.tensor.matmul(psc, wflat_s, xt_f[:, ds(cc * 512, 512)],
                                         start=True, stop=True)
                        nc.scalar.activation(out=convout[:, ds(cc * 512, 512)], in_=psc,
                                             func=AF.Copy)

                    # ---- E = tanh(dense(conv) + pm + q) ; energies = v . E
                    tanhE = wk.tile([128, BS * T_ENC], BF16, tag="tanhE")
                    e4 = wk.tile([BS, T_ENC], F32, tag="e4")
                    for h in range(2):
                        pe = psE.tile([128, 1536], F32, tag="pse")
                        for c3 in range(3):
                            sl = ds(h * 1536 + c3 * 512, 512)
                            lsl = ds(c3 * 512, 512)
                            nc.tensor.matmul(pe[:, lsl], ldT_s, convout[:, sl],
                                             start=True, stop=False)
                            nc.tensor.matmul(pe[:, lsl], ident_bf, pm_s[:, sl],
                                             start=False, stop=True)
                        for bb in range(2):
                            b_ = h * 2 + bb
                            nc.scalar.activation(
                                out=tanhE[:, ds(b_ * T_ENC, T_ENC)],
                                in_=pe[:, ds(bb * T_ENC, T_ENC)],
                                func=AF.Tanh, bias=q_sb[:, ds(b_, 1)], scale=1.0)
                        pev = psE.tile([1, 1536], F32, tag="pse")
                        for c3 in range(3):
                            nc.tensor.matmul(pev[:, ds(c3 * 512, 512)], v_s,
                                             tanhE[:, ds(h * 1536 + c3 * 512, 512)],
                                             start=True, stop=True)
                        ehalf = wk.tile([1, 1536], F32, tag="ehalf")
                        nc.scalar.activation(out=ehalf, in_=pev, func=AF.Copy)
                        nc.gpsimd.dma_start(out=e4[ds(h * 2, 2), :], in_=ehalf)

                    # ---- masked softmax over T_ENC (fp32, 4 partitions)
                    nc.vector.tensor_add(e4, e4, maskadd_s)
                    mx = wk.tile([BS, 1], F32, tag="mx")
                    nc.vector.reduce_max(mx, e4, axis=mybir.AxisListType.X)
                    nmx = wk.tile([BS, 1], F32, tag="nmx")
                    nc.vector.tensor_scalar_mul(nmx, mx, -1.0)
                    nc.scalar.activation(out=e4, in_=e4, func=AF.Exp, bias=nmx, scale=1.0)
                    sm = wk.tile([BS, 1], F32, tag="sm")
                    nc.vector.reduce_sum(sm, e4, axis=mybir.AxisListType.X)
                    rs = wk.tile([BS, 1], F32, tag="rs")
                    nc.vector.reciprocal(rs, sm)
                    aw_sb = wk.tile([BS, T_ENC], F32, tag="awsb")
                    nc.vector.tensor_scalar(aw_sb, e4, rs, None, mybir.AluOpType.mult)
                    nc.vector.tensor_add(awc_sb, awc_sb, aw_sb)
                    nc.gpsimd.dma_start(out=align_o[ds(iv, 1), :, :], in_=aw_sb)
                    aw_bf = wk.tile([BS, T_ENC], BF16, tag="awbf")
                    awc_bf = wk.tile([BS, T_ENC], BF16, tag="awcbf")
                    nc.vector.tensor_copy(aw_bf, aw_sb)
                    nc.vector.tensor_copy(awc_bf, awc_sb)
                    nc.gpsimd.dma_start(
                        out=bass.AP(tensor=awpad_d.tensor, offset=awpad_d.offset + 15,
                                    ap=[[PADW, BS], [1, T_ENC]]), in_=aw_bf)
                    nc.gpsimd.dma_start(
                        out=bass.AP(tensor=awpad_d.tensor,
                                    offset=awpad_d.offset + BS * PADW + 15,
                                    ap=[[PADW, BS], [1, T_ENC]]), in_=awc_bf)

                    # ---- awT via transposes, then ctx = aw @ memory
                    awT = wk.tile([128, 6, BS], BF16, tag="awT")
                    for j in range(6):
                        pst = ps1.tile([128, BS], F32, tag="ps_s")
                        nc.tensor.transpose(pst, aw_sb[:, ds(j * 128, 128)], ident[0:BS, 0:BS])
                        nc.vector.tensor_copy(awT[:, j, :], pst)
                    ctx_bf = wk.tile([BS, BS * ENC], BF16, tag="ctxbf")
                    for cc in range(4):
                        pc = ps1.tile([BS, 512], F32, tag="ps_s")
                        for kt in range(6):
                            nc.tensor.matmul(pc, awT[:, kt, :],
                                             memT2_s[:, kt, ds(cc * 512, 512)],
                                             start=(kt == 0), stop=(kt == 5))
                        nc.scalar.activation(out=ctx_bf[:, ds(cc * 512, 512)], in_=pc,
                                             func=AF.Copy)
                    for b_ in range(BS):
                        nc.gpsimd.dma_start(out=ctx_d[ds(iv, 1), ds(b_, 1), :],
                                            in_=ctx_bf[ds(b_, 1), ds(b_ * ENC, ENC)])
                    for kt in range(4):
                        nc.gpsimd.dma_start(
                            out=actT[:, kt, :],
                            in_=ctx_d[ds(iv, 1), :, ds(kt * 128, 128)].rearrange(
                                "o b dm -> (o dm) b"))

            # ============================================ phase 2: G2 + pre_mel/gate
            with (
                tc.tile_pool(name="c2", bufs=1) as c2,
                tc.tile_pool(name="wk2", bufs=3) as wk2,
                tc.tile_pool(name="ps2", bufs=2, space="PSUM") as ps2,
            ):
                wdi_s = c2.tile([128, 12, GD], BF16)
                pctx_s = c2.tile([128, 4, N_MELS], BF16)
                gctx_s = c2.tile([128, 4, 1], BF16)
                nc.gpsimd.dma_start(out=wdi_s, in_=wdi[:, :, :])
                nc.gpsimd.dma_start(out=pctx_s, in_=projctx[:, :, :])
                nc.gpsimd.dma_start(out=gctx_s, in_=gatectx[:, :, :])
                NROW = td * BS
                for nt in range(NT):
                    nrows = min(128, NROW - nt * 128)
                    dts = []
                    for kt in range(12):
                        dt_ = wk2.tile([128, 128], BF16, tag=f"dt{kt}")
                        if kt < 8:
                            src = ah_d.rearrange("t b k -> (t b) k")[ds(nt * 128, nrows),
                                                                     ds(kt * 128, 128)]
                        else:
                            src = ctx_d.rearrange("t b k -> (t b) k")[
                                ds(nt * 128, nrows), ds((kt - 8) * 128, 128)]
                        nc.sync.dma_start(out=dt_[:, 0:nrows], in_=src, transpose=True)
                        dts.append(dt_)
                    for gc in range(8):
                        ps = ps2.tile([128, 512], F32, tag="p2")
                        for kt in range(12):
                            nc.tensor.matmul(ps[:nrows, :], dts[kt][:, 0:nrows],
                                             wdi_s[:, kt, ds(gc * 512, 512)],
                                             start=(kt == 0), stop=(kt == 11))
                        ob = wk2.tile([128, 512], BF16, tag="ob")
                        nc.scalar.activation(out=ob[:nrows, :], in_=ps[:nrows, :], func=AF.Copy)
                        nc.gpsimd.dma_start(
                            out=g2_d.rearrange("t b g -> (t b) g")[ds(nt * 128, nrows),
                                                                   ds(gc * 512, 512)],
                            in_=ob[:nrows, :])
                    # pre_mel / pre_gate from ctx k-tiles (dts[8:12])
                    psm = ps2.tile([128, N_MELS], F32, tag="pm2")
                    psg = ps2.tile([128, 1], F32, tag="pg2")
                    for kt in range(4):
                        nc.tensor.matmul(psm[:nrows, :], dts[8 + kt][:, 0:nrows],
                                         pctx_s[:, kt, :], start=(kt == 0), stop=(kt == 3))
                        nc.tensor.matmul(psg[:nrows, :], dts[8 + kt][:, 0:nrows],
                                         gctx_s[:, kt, :], start=(kt == 0), stop=(kt == 3))
                    obm = wk2.tile([128, N_MELS], F32, tag="obm")
                    obg = wk2.tile([128, 1], F32, tag="obg")
                    nc.scalar.activation(out=obm[:nrows, :], in_=psm[:nrows, :], func=AF.Copy)
                    nc.scalar.activation(out=obg[:nrows, :], in_=psg[:nrows, :], func=AF.Copy)
                    nc.gpsimd.dma_start(
                        out=premel_d.rearrange("t b g -> (t b) g")[ds(nt * 128, nrows), :],
                        in_=obm[:nrows, :])
                    nc.gpsimd.dma_start(
                        out=pregate_d.rearrange("t b g -> (t b) g")[ds(nt * 128, nrows), :],
                        in_=obg[:nrows, :])

            # ============================================ phase 3: decoder scan
            with (
                tc.tile_pool(name="c3", bufs=1) as c3,
                tc.tile_pool(name="st3", bufs=1) as st3,
                tc.tile_pool(name="wk3", bufs=1) as wk3,
                tc.tile_pool(name="ps3", bufs=4, space="PSUM") as ps3,
            ):
                wd_s = c3.tile([128, 8, GD], BF16)
                pdh_s = c3.tile([128, 8, N_MELS], BF16)
                gdh_s = c3.tile([128, 8, 1], BF16)
                ident3 = c3.tile([128, 128], F32)
                ident3_bf = c3.tile([128, 128], BF16)
                nc.gpsimd.dma_start(out=wd_s, in_=wd[:, :, :])
                nc.gpsimd.dma_start(out=pdh_s, in_=projdh[:, :, :])
                nc.gpsimd.dma_start(out=gdh_s, in_=gatedh[:, :, :])
                make_identity(nc, ident3)
                nc.vector.tensor_copy(ident3_bf, ident3)

                dhT = st3.tile([128, 8, BS], BF16)
                dc_sb = st3.tile([BS, RNN], F32)
                nc.vector.memset(dhT, 0.0)
                nc.vector.memset(dc_sb, 0.0)

                with tc.For_i(0, td) as iv:
                    g2_t = wk3.tile([BS, GD], BF16, tag="g2t")
                    nc.gpsimd.dma_start(out=g2_t, in_=g2_d[ds(iv, 1), :, :])
                    mdh_t = wk3.tile([BS, RNN], BF16, tag="mdh")
                    nc.gpsimd.dma_start(out=mdh_t, in_=mdh_i[ds(iv, 1), :, :])
                    pmel_t = wk3.tile([BS, N_MELS], F32, tag="pmel")
                    nc.gpsimd.dma_start(out=pmel_t, in_=premel_d[ds(iv, 1), :, :])
                    pgate_t = wk3.tile([BS, 1], F32, tag="pgate")
                    nc.gpsimd.dma_start(out=pgate_t, in_=pregate_d[ds(iv, 1), :, :])

                    dh_b = wk3.tile([BS, RNN], F32, tag="dhb")
                    for qd in range(2):
                        sg = {}
                        for ti, tn in enumerate("ifog"):
                            ps = ps3.tile([BS, 512], F32, tag="p3")
                            gc = qd * 4 + ti
                            nc.tensor.matmul(ps, ident3_bf[0:BS, 0:BS],
                                             g2_t[:, ds(gc * 512, 512)], start=True, stop=False)
                            for kt in range(8):
                                nc.tensor.matmul(ps, dhT[:, kt, :],
                                                 wd_s[:, kt, ds(gc * 512, 512)],
                                                 start=False, stop=(kt == 7))
                            sg[tn] = wk3.tile([BS, 512], F32, tag=f"sg{tn}", name=f"sg{tn}3")
                            nc.scalar.activation(out=sg[tn], in_=ps,
                                                 func=(AF.Tanh if tn == "g" else AF.Sigmoid))
                        qsl = ds(qd * 512, 512)
                        t1 = wk3.tile([BS, 512], F32, tag="t1")
                        t2 = wk3.tile([BS, 512], F32, tag="t2")
                        nc.vector.tensor_mul(t1, sg["f"], dc_sb[:, qsl])
                        nc.vector.tensor_mul(t2, sg["i"], sg["g"])
                        nc.vector.tensor_add(dc_sb[:, qsl], t1, t2)
                        tnc = wk3.tile([BS, 512], F32, tag="tnc")
                        nc.scalar.activation(out=tnc, in_=dc_sb[:, qsl], func=AF.Tanh)
                        nc.vector.tensor_mul(sg["o"], sg["o"], mdh_t[:, qsl])
                        nc.vector.tensor_mul(dh_b[:, qsl], sg["o"], tnc)
                    for j in range(8):
                        pst = ps3.tile([128, BS], F32, tag="p3t")
                        nc.tensor.transpose(pst, dh_b[:, ds(j * 128, 128)], ident3[0:BS, 0:BS])
                        nc.vector.tensor_copy(dhT[:, j, :], pst)
                    # mel + gate
                    psm = ps3.tile([BS, N_MELS], F32, tag="p3t")
                    nc.tensor.matmul(psm, ident3[0:BS, 0:BS], pmel_t, start=True, stop=False)
                    for j in range(8):
                        nc.tensor.matmul(psm, dhT[:, j, :], pdh_s[:, j, :],
                                         start=False, stop=(j == 7))
                    mel_sb = wk3.tile([BS, N_MELS], F32, tag="melsb")
                    nc.scalar.activation(out=mel_sb, in_=psm, func=AF.Copy)
                    nc.gpsimd.dma_start(out=mel_o[ds(iv, 1), :, :], in_=mel_sb)
                    psg = ps3.tile([BS, 1], F32, tag="p3t")
                    nc.tensor.matmul(psg, ident3[0:BS, 0:BS], pgate_t, start=True, stop=False)
                    for j in range(8):
                        nc.tensor.matmul(psg, dhT[:, j, :], gdh_s[:, j, :],
                                         start=False, stop=(j == 7))
                    gate_sb = wk3.tile([BS, 1], F32, tag="gatesb")
                    nc.scalar.activation(out=gate_sb, in_=psg, func=AF.Sigmoid)
                    nc.gpsimd.dma_start(out=gate_o[ds(iv, 1), :], in_=gate_sb)

    nc.compile()
    return nc


# ---------------------------------------------------------------- entry point

_CACHE = {}


def kernel(memory, decoder_inputs, memory_lengths, params, td=T_DEC, trace=False):
    in_maps = host_prep(memory, decoder_inputs, memory_lengths, params, td=td)
    if td not in _CACHE:
        _CACHE[td] = build_nc(td)
    nc = _CACHE[td]
    res = run_bass_kernel_spmd(nc, in_maps, core_ids=list(range(NCORES)), trace=trace)
    mels, gates, aligns = [], [], []
    for c in range(NCORES):
        r = res.results[c]
        mels.append(r["mel_o"])
        gates.append(r["gate_o"])
        aligns.append(r["align_o"])
    mel = np.concatenate(mels, axis=1)      # (td,B,80)
    gate = np.concatenate(gates, axis=1)    # (td,B)
    align = np.concatenate(aligns, axis=1)  # (td,B,TE)
    out = (np.ascontiguousarray(np.transpose(mel, (1, 2, 0))),
           np.ascontiguousarray(np.transpose(gate, (1, 0))),
           np.ascontiguousarray(np.transpose(align, (1, 0, 2))))
    kernel._last_result = res
    return out


# revision 29
# speedup vs baseline: 63.0600x; 63.0600x over previous
"""Trainium2 Bass kernel for the Tacotron2-style decoder (nn_Decoder_70377334112524).

Strategy: pure data parallel over batch (B=32 -> 4 per core x 8 cores).
Per core, the computation is restructured into 4 phases:
  0: prenet over all frames + G1a = prenet_out @ Wa_ih[:, :256].T  (batched matmuls)
  1: sequential scan t=0..T-1 of attention-LSTM + location-sensitive attention
  2: G2 = [ah|ctx] @ Wd_ih.T batched; pre_mel/pre_gate = ctx @ proj/gate ctx-part
  3: sequential scan of decoder-LSTM + projections (only W_hh recurrent matmul)

Weights live in SBUF (bf16); all accumulation fp32 in PSUM; softmax/state fp32.
Dropout masks are data-independent: reproduced exactly on host with JAX threefry
(CPU backend), matching the reference's per-step bernoulli draws inside scan.
"""

import numpy as np
import ml_dtypes

import concourse.bass as bass
import concourse.tile as tile
from concourse import bacc, mybir
from concourse.bass import ds
from concourse.bass_utils import run_bass_kernel_spmd
from concourse.masks import make_identity

AF = mybir.ActivationFunctionType
F32 = mybir.dt.float32
BF16 = mybir.dt.bfloat16

B, T_ENC, T_DEC = 32, 768, 400
N_MELS, PRENET, ENC, RNN, ATT, LOC_F, LOC_K = 80, 256, 512, 1024, 128, 32, 31
NCORES = 8
BS = B // NCORES  # 4
GD = 4 * RNN      # 4096
bf = ml_dtypes.bfloat16

# gate-row permutation: per 512-quad [i|f|o|g] so the cell updates chunkwise
_PERM = np.concatenate([np.concatenate(
    [np.arange(0, 512) + qd * 512,          # i
     np.arange(1024, 1536) + qd * 512,      # f
     np.arange(3072, 3584) + qd * 512,      # o
     np.arange(2048, 2560) + qd * 512])     # g
    for qd in (0, 1)])


# ---------------------------------------------------------------- host helpers

def _masks(td):
    """Exact dropout masks as the reference draws them (threefry, forced CPU)."""
    import jax

    def fn():
        key = jax.random.key(42)
        k1, k2, k3 = jax.random.split(key, 3)
        m1 = jax.random.bernoulli(k1, 0.5, (T_DEC + 1, B, PRENET))
        m2 = jax.random.bernoulli(k2, 0.5, (T_DEC + 1, B, PRENET))
        sk = jax.random.split(k3, T_DEC * 2).reshape(T_DEC, 2)

        def mstep(c, ks):
            return c, (jax.random.bernoulli(ks[0], 0.9, (B, RNN)),
                       jax.random.bernoulli(ks[1], 0.9, (B, RNN)))
        _, (mah, mdh) = jax.lax.scan(mstep, 0, sk)
        return m1, m2, mah, mdh

    m1, m2, mah, mdh = jax.jit(fn, backend="cpu")()
    m1 = np.asarray(m1[:td]).astype(np.float32) * 2.0
    m2 = np.asarray(m2[:td]).astype(np.float32) * 2.0
    mah = np.asarray(mah[:td]).astype(np.float32) / 0.9
    mdh = np.asarray(mdh[:td]).astype(np.float32) / 0.9
    return m1, m2, mah, mdh


def _ktiles(wT, kt):
    """(K, N) -> (128, kt, N) with [p, i, n] = wT[i*128+p, n]."""
    K, N = wT.shape
    assert K == kt * 128
    return np.ascontiguousarray(np.transpose(wT.reshape(kt, 128, N), (1, 0, 2)))


def host_prep(memory, decoder_inputs, memory_lengths, params, td=T_DEC):
    p = {k: np.asarray(v, np.float32) for k, v in params.items()}
    memory = np.asarray(memory, np.float32)
    decoder_inputs = np.asarray(decoder_inputs, np.float32)
    memory_lengths = np.asarray(memory_lengths)

    for bn in ("pre_b1", "pre_b2", "arnn_b_ih", "arnn_b_hh", "drnn_b_ih",
               "drnn_b_hh", "proj_b", "gate_b"):
        assert np.abs(p[bn]).max() == 0.0, f"nonzero bias {bn} not supported"

    m1, m2, mah, mdh = _masks(td)

    # shared (replicated) weight tensors
    wa = np.concatenate([p["arnn_w_ih"][:, PRENET:], p["arnn_w_hh"]], axis=1)[_PERM]
    shared = {
        "w1T": p["pre_w1"].T.astype(bf),                              # (80,256)
        "w2T": _ktiles(p["pre_w2"].T, 2).astype(bf),                  # (128,2,256)
        "wpT": _ktiles(p["arnn_w_ih"][:, :PRENET][_PERM].T, 2).astype(bf),  # (128,2,4096)
        "wa": _ktiles(wa.T, 12).astype(bf),                           # (128,12,4096)
        "wq": _ktiles(p["attn_q_w"].T, 8).astype(bf),                 # (128,8,128)
        "wflat": np.transpose(p["loc_conv_w"], (1, 2, 0)).reshape(62, LOC_F).astype(bf),
        "ldT": p["loc_dense_w"].T.astype(bf),                         # (32,128)
        "vT": p["attn_v"].T.astype(bf),                               # (128,1)
        "wd": _ktiles(p["drnn_w_hh"][_PERM].T, 8).astype(bf),         # (128,8,4096)
        "wdi": _ktiles(p["drnn_w_ih"][_PERM].T, 12).astype(bf),       # (128,12,4096)
        "projdh": _ktiles(p["proj_w"][:, :RNN].T, 8).astype(bf),      # (128,8,80)
        "projctx": _ktiles(p["proj_w"][:, RNN:].T, 4).astype(bf),     # (128,4,80)
        "gatedh": _ktiles(p["gate_w"][:, :RNN].T, 8).astype(bf),      # (128,8,1)
        "gatectx": _ktiles(p["gate_w"][:, RNN:].T, 4).astype(bf),     # (128,4,1)
    }

    pm = np.einsum("btd,fd->btf", memory, p["attn_mem_w"])            # (B,TE,128)
    x = np.concatenate([np.zeros((1, B, N_MELS), np.float32),
                        np.transpose(decoder_inputs, (2, 0, 1))], 0)[:td]
    maskadd = np.where(np.arange(T_ENC)[None, :] >= memory_lengths[:, None],
                       np.float32(-1e30), np.float32(0.0)).astype(np.float32)

    in_maps = []
    for c in range(NCORES):
        sl = slice(c * BS, (c + 1) * BS)
        mem_c = memory[sl]                                            # (4,TE,512)
        d = dict(shared)
        d["xT"] = np.ascontiguousarray(np.transpose(x[:, sl, :], (2, 0, 1))).astype(bf)  # (80,td,4)
        d["m1T"] = np.ascontiguousarray(
            np.transpose(m1[:, sl, :].reshape(td, BS, 2, 128), (3, 2, 0, 1))).astype(bf)  # (128,2,td,4)
        d["m2T"] = np.ascontiguousarray(
            np.transpose(m2[:, sl, :].reshape(td, BS, 2, 128), (3, 2, 0, 1))).astype(bf)
        d["mah"] = mah[:, sl, :].astype(bf)                           # (td,4,1024)
        d["mdh"] = mdh[:, sl, :].astype(bf)
        d["pm"] = np.ascontiguousarray(np.transpose(pm[sl], (2, 0, 1))).astype(bf)  # (128,4,TE)
        d["memT2"] = np.ascontiguousarray(
            np.transpose(mem_c.reshape(BS, 6, 128, ENC), (2, 1, 0, 3))).astype(bf)  # (128,6,4,512)
        d["maskadd"] = np.ascontiguousarray(maskadd[sl])              # (4,TE)
        in_maps.append(d)
    return in_maps


# ---------------------------------------------------------------- bass builder

def build_nc(td=T_DEC):
    nc = bacc.Bacc(None, target_bir_lowering=False)

    def DI(name, shape, dt):
        return nc.dram_tensor(name, list(shape), dt, kind="ExternalInput")

    # inputs
    w1T = DI("w1T", (80, 256), BF16)
    w2T = DI("w2T", (128, 2, 256), BF16)
    wpT = DI("wpT", (128, 2, GD), BF16)
    wa = DI("wa", (128, 12, GD), BF16)
    wq = DI("wq", (128, 8, 128), BF16)
    wflat = DI("wflat", (62, LOC_F), BF16)
    ldT = DI("ldT", (LOC_F, 128), BF16)
    vT = DI("vT", (128, 1), BF16)
    wd = DI("wd", (128, 8, GD), BF16)
    wdi = DI("wdi", (128, 12, GD), BF16)
    projdh = DI("projdh", (128, 8, N_MELS), BF16)
    projctx = DI("projctx", (128, 4, N_MELS), BF16)
    gatedh = DI("gatedh", (128, 8, 1), BF16)
    gatectx = DI("gatectx", (128, 4, 1), BF16)
    xT = DI("xT", (80, td, BS), BF16)
    m1T = DI("m1T", (128, 2, td, BS), BF16)
    m2T = DI("m2T", (128, 2, td, BS), BF16)
    mah_i = DI("mah", (td, BS, RNN), BF16)
    mdh_i = DI("mdh", (td, BS, RNN), BF16)
    pm_i = DI("pm", (128, BS, T_ENC), BF16)
    memT2_i = DI("memT2", (128, 6, BS, ENC), BF16)
    maskadd_i = DI("maskadd", (BS, T_ENC), F32)

    # outputs
    mel_o = nc.dram_tensor("mel_o", [td, BS, N_MELS], F32, kind="ExternalOutput")
    gate_o = nc.dram_tensor("gate_o", [td, BS], F32, kind="ExternalOutput")
    align_o = nc.dram_tensor("align_o", [td, BS, T_ENC], F32, kind="ExternalOutput")

    NT = (td * BS + 127) // 128  # n-tiles over (t,b) rows

    with tile.TileContext(nc) as tc:
        with tc.tile_pool(name="dram", bufs=1, space="DRAM") as dpool:
            g1a_d = dpool.tile([td, BS, GD], BF16)
            awpad_d = dpool.tile([2 * BS, T_ENC + 2 * (LOC_K // 2)], BF16)
            ah_d = dpool.tile([td, BS, RNN], BF16)
            ctx_d = dpool.tile([td, BS, ENC], BF16)
            g2_d = dpool.tile([td, BS, GD], BF16)
            premel_d = dpool.tile([td, BS, N_MELS], F32)
            pregate_d = dpool.tile([td, BS, 1], F32)

            # ============================================ phase 0: prenet + G1a
            with (
                tc.tile_pool(name="p0c", bufs=1) as p0c,
                tc.tile_pool(name="p0w", bufs=3) as p0w,
                tc.tile_pool(name="p0ps", bufs=4, space="PSUM") as p0ps,
            ):
                w1s = p0c.tile([80, 256], BF16)
                w2s = p0c.tile([128, 2, 256], BF16)
                wps = p0c.tile([128, 2, GD], BF16)
                xTs = p0c.tile([80, td * BS], BF16)
                m1s = p0c.tile([128, 2, td * BS], BF16)
                m2s = p0c.tile([128, 2, td * BS], BF16)
                h1 = p0c.tile([128, 2, td * BS], BF16)
                h2 = p0c.tile([128, 2, td * BS], BF16)
                nc.gpsimd.dma_start(out=w1s, in_=w1T[:, :])
                nc.gpsimd.dma_start(out=w2s, in_=w2T[:, :, :])
                nc.gpsimd.dma_start(out=wps, in_=wpT[:, :, :])
                nc.gpsimd.dma_start(out=xTs, in_=xT.rearrange("c t b -> c (t b)"))
                nc.gpsimd.dma_start(out=m1s, in_=m1T.rearrange("p c t b -> p c (t b)"))
                nc.gpsimd.dma_start(out=m2s, in_=m2T.rearrange("p c t b -> p c (t b)"))

                NROW = td * BS
                nch = (NROW + 511) // 512
                for mc in range(2):
                    for nn in range(nch):
                        nsl = ds(nn * 512, min(512, NROW - nn * 512))
                        ps = p0ps.tile([128, 512], F32)
                        nc.tensor.matmul(ps[:, 0:nsl.size], w1s[:, ds(mc * 128, 128)],
                                         xTs[:, nsl], start=True, stop=True)
                        nc.scalar.activation(out=h1[:, mc, nsl], in_=ps[:, 0:nsl.size],
                                             func=AF.Relu)
                        nc.vector.tensor_mul(h1[:, mc, nsl], h1[:, mc, nsl], m1s[:, mc, nsl])
                for mc in range(2):
                    for nn in range(nch):
                        nsl = ds(nn * 512, min(512, NROW - nn * 512))
                        ps = p0ps.tile([128, 512], F32)
                        for kt in range(2):
                            nc.tensor.matmul(ps[:, 0:nsl.size], w2s[:, kt, ds(mc * 128, 128)],
                                             h1[:, kt, nsl], start=(kt == 0), stop=(kt == 1))
                        nc.scalar.activation(out=h2[:, mc, nsl], in_=ps[:, 0:nsl.size],
                                             func=AF.Relu)
                        nc.vector.tensor_mul(h2[:, mc, nsl], h2[:, mc, nsl], m2s[:, mc, nsl])
                # G1a = pn @ Wp.T  -> (td,4,4096) bf16
                for nt in range(NT):
                    nrows = min(128, NROW - nt * 128)
                    for gc in range(8):
                        ps = p0ps.tile([128, 512], F32)
                        for kt in range(2):
                            nc.tensor.matmul(ps[:nrows, :], h2[:, kt, ds(nt * 128, nrows)],
                                             wps[:, kt, ds(gc * 512, 512)],
                                             start=(kt == 0), stop=(kt == 1))
                        ob = p0w.tile([128, 512], BF16)
                        nc.scalar.activation(out=ob[:nrows, :], in_=ps[:nrows, :], func=AF.Copy)
                        nc.gpsimd.dma_start(
                            out=g1a_d.rearrange("t b g -> (t b) g")[ds(nt * 128, nrows),
                                                                    ds(gc * 512, 512)],
                            in_=ob[:nrows, :])

            # ============================================ phase 1: attention scan
            with (
                tc.tile_pool(name="c1", bufs=1) as c1,
                tc.tile_pool(name="st", bufs=1) as st,
                tc.tile_pool(name="wk", bufs=1) as wk,
                tc.tile_pool(name="ps1", bufs=4, space="PSUM") as ps1,
                tc.tile_pool(name="psE", bufs=1, space="PSUM") as psE,
            ):
                wa_s = c1.tile([128, 12, GD], BF16)
                memT2_s = c1.tile([128, 6, BS * ENC], BF16)
                pm_s = c1.tile([128, BS * T_ENC], BF16)
                wq_s = c1.tile([128, 8, 128], BF16)
                wflat_s = c1.tile([62, LOC_F], BF16)
                ldT_s = c1.tile([LOC_F, 128], BF16)
                v_s = c1.tile([128, 1], BF16)
                maskadd_s = c1.tile([BS, T_ENC], F32)
                ident = c1.tile([128, 128], F32)
                ident_bf = c1.tile([128, 128], BF16)
                nc.gpsimd.dma_start(out=wa_s, in_=wa[:, :, :])
                nc.gpsimd.dma_start(out=memT2_s, in_=memT2_i.rearrange("p k b d -> p k (b d)"))
                nc.gpsimd.dma_start(out=pm_s, in_=pm_i.rearrange("p b t -> p (b t)"))
                nc.gpsimd.dma_start(out=wq_s, in_=wq[:, :, :])
                nc.gpsimd.dma_start(out=wflat_s, in_=wflat[:, :])
                nc.gpsimd.dma_start(out=ldT_s, in_=ldT[:, :])
                nc.gpsimd.dma_start(out=v_s, in_=vT[:, :])
                nc.gpsimd.dma_start(out=maskadd_s, in_=maskadd_i[:, :])
                make_identity(nc, ident)
                nc.vector.tensor_copy(ident_bf, ident)

                # persistent state
                actT = st.tile([128, 12, BS], BF16)   # [ctx(4) | ah(8)] feature-T
                c_sb = st.tile([BS, RNN], F32)
                awc_sb = st.tile([BS, T_ENC], F32)
                zpad = st.tile([2 * BS, T_ENC + 30], BF16)
                nc.vector.memset(actT, 0.0)
                nc.vector.memset(c_sb, 0.0)
                nc.vector.memset(awc_sb, 0.0)
                nc.vector.memset(zpad, 0.0)
                nc.gpsimd.dma_start(out=awpad_d, in_=zpad)

                PADW = T_ENC + 30

                with tc.For_i(0, td) as iv:
                    g1a_t = wk.tile([BS, GD], BF16, tag="g1a")
                    nc.gpsimd.dma_start(out=g1a_t, in_=g1a_d[ds(iv, 1), :, :])
                    mah_t = wk.tile([BS, RNN], BF16, tag="mah")
                    nc.gpsimd.dma_start(out=mah_t, in_=mah_i[ds(iv, 1), :, :])
                    xt = wk.tile([62, BS, T_ENC], BF16, tag="xt")
                    for i2 in range(2):
                        nc.gpsimd.dma_start(
                            out=xt[ds(i2 * 31, 31), :, :],
                            in_=bass.AP(tensor=awpad_d.tensor,
                                        offset=awpad_d.offset + i2 * BS * PADW,
                                        ap=[[1, 31], [PADW, BS], [1, T_ENC]]))

                    # ---- attention LSTM gates: per 512-quad [i|f|o|g] chunks
                    ah_b = wk.tile([BS, RNN], F32, tag="ahb")
                    for qd in range(2):
                        sg = {}
                        for ti, tn in enumerate("ifog"):
                            ps = ps1.tile([BS, 512], F32, tag="ps_s")
                            gc = qd * 4 + ti
                            nc.tensor.matmul(ps, ident_bf[0:BS, 0:BS],
                                             g1a_t[:, ds(gc * 512, 512)], start=True, stop=False)
                            for kt in range(12):
                                nc.tensor.matmul(ps, actT[:, kt, :],
                                                 wa_s[:, kt, ds(gc * 512, 512)],
                                                 start=False, stop=(kt == 11))
                            sg[tn] = wk.tile([BS, 512], F32, tag=f"sg{tn}", name=f"sg{tn}")
                            nc.scalar.activation(out=sg[tn], in_=ps,
                                                 func=(AF.Tanh if tn == "g" else AF.Sigmoid))
                        qsl = ds(qd * 512, 512)
                        t1 = wk.tile([BS, 512], F32, tag="t1")
                        t2 = wk.tile([BS, 512], F32, tag="t2")
                        nc.vector.tensor_mul(t1, sg["f"], c_sb[:, qsl])
                        nc.vector.tensor_mul(t2, sg["i"], sg["g"])
                        nc.vector.tensor_add(c_sb[:, qsl], t1, t2)
                        tnc = wk.tile([BS, 512], F32, tag="t1", name="tnc")
                        nc.scalar.activation(out=tnc, in_=c_sb[:, qsl], func=AF.Tanh)
                        nc.vector.tensor_mul(sg["o"], sg["o"], mah_t[:, qsl])
                        nc.vector.tensor_mul(ah_b[:, qsl], sg["o"], tnc)
                    ah_bf = wk.tile([BS, RNN], BF16, tag="ahbf")
                    nc.vector.tensor_copy(ah_bf, ah_b)
                    nc.gpsimd.dma_start(out=ah_d[ds(iv, 1), :, :], in_=ah_bf)
                    # transposes ah -> actT[:, 4+j, :]
                    for j in range(8):
                        pst = ps1.tile([128, BS], F32, tag="ps_s")
                        nc.tensor.transpose(pst, ah_b[:, ds(j * 128, 128)], ident[0:BS, 0:BS])
                        nc.vector.tensor_copy(actT[:, 4 + j, :], pst)

                    # ---- q = ah @ Wq.T  (feature-T out)
                    psq = ps1.tile([128, BS], F32, tag="ps_s")
                    for j in range(8):
                        nc.tensor.matmul(psq, wq_s[:, j, :], actT[:, 4 + j, :],
                                         start=(j == 0), stop=(j == 7))
                    q_sb = wk.tile([128, BS], F32, tag="qsb")
                    nc.scalar.activation(out=q_sb, in_=psq, func=AF.Copy)

                    # ---- conv (as matmul from shifted windows) -> convout (32,(b,t))
                    convout = wk.tile([LOC_F, BS * T_ENC], BF16, tag="conv")
                    xt_f = xt.rearrange("p b t -> p (b t)")
                    for cc in range(6):
                        psc = ps1.tile([LOC_F, 512], F32, tag="ps_s")
                        nc.tensor.matmul(psc, wflat_s, xt_f[:, ds(cc * 512, 512)],
                                         start=True, stop=True)
                        nc.scalar.activation(out=convout[:, ds(cc * 512, 512)], in_=psc,
                                             func=AF.Copy)

                    # ---- E = tanh(dense(conv) + pm + q) ; energies = v . E
                    tanhE = wk.tile([128, BS * T_ENC], BF16, tag="tanhE")
                    e4 = wk.tile([BS, T_ENC], F32, tag="e4")
                    for h in range(2):
                        pe = psE.tile([128, 1536], F32, tag="pse")
                        for c3 in range(3):
                            sl = ds(h * 1536 + c3 * 512, 512)
                            lsl = ds(c3 * 512, 512)
                            nc.tensor.matmul(pe[:, lsl], ldT_s, convout[:, sl],
                                             start=True, stop=False)
                            nc.tensor.matmul(pe[:, lsl], ident_bf, pm_s[:, sl],
                                             start=False, stop=True)
                        for bb in range(2):
                            b_ = h * 2 + bb
                            nc.scalar.activation(
                                out=tanhE[:, ds(b_ * T_ENC, T_ENC)],
                                in_=pe[:, ds(bb * T_ENC, T_ENC)],
                                func=AF.Tanh, bias=q_sb[:, ds(b_, 1)], scale=1.0)
                        pev = psE.tile([1, 1536], F32, tag="pse")
                        for c3 in range(3):
                            nc.tensor.matmul(pev[:, ds(c3 * 512, 512)], v_s,
                                             tanhE[:, ds(h * 1536 + c3 * 512, 512)],
                                             start=True, stop=True)
                        ehalf = wk.tile([1, 1536], F32, tag="ehalf")
                        nc.scalar.activation(out=ehalf, in_=pev, func=AF.Copy)
                        nc.gpsimd.dma_start(out=e4[ds(h * 2, 2), :], in_=ehalf)

                    # ---- masked softmax over T_ENC (fp32, 4 partitions)
                    nc.vector.tensor_add(e4, e4, maskadd_s)
                    mx = wk.tile([BS, 1], F32, tag="mx")
                    nc.vector.reduce_max(mx, e4, axis=mybir.AxisListType.X)
                    nmx = wk.tile([BS, 1], F32, tag="nmx")
                    nc.vector.tensor_scalar_mul(nmx, mx, -1.0)
                    nc.scalar.activation(out=e4, in_=e4, func=AF.Exp, bias=nmx, scale=1.0)
                    sm = wk.tile([BS, 1], F32, tag="sm")
                    nc.vector.reduce_sum(sm, e4, axis=mybir.AxisListType.X)
                    rs = wk.tile([BS, 1], F32, tag="rs")
                    nc.vector.reciprocal(rs, sm)
                    aw_sb = wk.tile([BS, T_ENC], F32, tag="awsb")
                    nc.vector.tensor_scalar(aw_sb, e4, rs, None, mybir.AluOpType.mult)
                    nc.vector.tensor_add(awc_sb, awc_sb, aw_sb)
                    nc.gpsimd.dma_start(out=align_o[ds(iv, 1), :, :], in_=aw_sb)
                    aw_bf = wk.tile([BS, T_ENC], BF16, tag="awbf")
                    awc_bf = wk.tile([BS, T_ENC], BF16, tag="awcbf")
                    nc.vector.tensor_copy(aw_bf, aw_sb)
                    nc.vector.tensor_copy(awc_bf, awc_sb)
                    nc.gpsimd.dma_start(
                        out=bass.AP(tensor=awpad_d.tensor, offset=awpad_d.offset + 15,
                                    ap=[[PADW, BS], [1, T_ENC]]), in_=aw_bf)
                    nc.gpsimd.dma_start(
                        out=bass.AP(tensor=awpad_d.tensor,
                                    offset=awpad_d.offset + BS * PADW + 15,
                                    ap=[[PADW, BS], [1, T_ENC]]), in_=awc_bf)

                    # ---- awT via transposes, then ctx = aw @ memory
                    awT = wk.tile([128, 6, BS], BF16, tag="awT")
                    for j in range(6):
                        pst = ps1.tile([128, BS], F32, tag="ps_s")
                        nc.tensor.transpose(pst, aw_sb[:, ds(j * 128, 128)], ident[0:BS, 0:BS])
                        nc.vector.tensor_copy(awT[:, j, :], pst)
                    ctx_bf = wk.tile([BS, BS * ENC], BF16, tag="ctxbf")
                    for cc in range(4):
                        pc = ps1.tile([BS, 512], F32, tag="ps_s")
                        for kt in range(6):
                            nc.tensor.matmul(pc, awT[:, kt, :],
                                             memT2_s[:, kt, ds(cc * 512, 512)],
                                             start=(kt == 0), stop=(kt == 5))
                        nc.scalar.activation(out=ctx_bf[:, ds(cc * 512, 512)], in_=pc,
                                             func=AF.Copy)
                    for b_ in range(BS):
                        nc.gpsimd.dma_start(out=ctx_d[ds(iv, 1), ds(b_, 1), :],
                                            in_=ctx_bf[ds(b_, 1), ds(b_ * ENC, ENC)])
                    for kt in range(4):
                        nc.gpsimd.dma_start(
                            out=actT[:, kt, :],
                            in_=ctx_d[ds(iv, 1), :, ds(kt * 128, 128)].rearrange(
                                "o b dm -> (o dm) b"))

            # ============================================ phase 2: G2 + pre_mel/gate
            with (
                tc.tile_pool(name="c2", bufs=1) as c2,
                tc.tile_pool(name="wk2", bufs=3) as wk2,
                tc.tile_pool(name="ps2", bufs=2, space="PSUM") as ps2,
            ):
                wdi_s = c2.tile([128, 12, GD], BF16)
                pctx_s = c2.tile([128, 4, N_MELS], BF16)
                gctx_s = c2.tile([128, 4, 1], BF16)
                nc.gpsimd.dma_start(out=wdi_s, in_=wdi[:, :, :])
                nc.gpsimd.dma_start(out=pctx_s, in_=projctx[:, :, :])
                nc.gpsimd.dma_start(out=gctx_s, in_=gatectx[:, :, :])
                NROW = td * BS
                for nt in range(NT):
                    nrows = min(128, NROW - nt * 128)
                    dts = []
                    for kt in range(12):
                        dt_ = wk2.tile([128, 128], BF16, tag=f"dt{kt}")
                        if kt < 8:
                            src = ah_d.rearrange("t b k -> (t b) k")[ds(nt * 128, nrows),
                                                                     ds(kt * 128, 128)]
                        else:
                            src = ctx_d.rearrange("t b k -> (t b) k")[
                                ds(nt * 128, nrows), ds((kt - 8) * 128, 128)]
                        nc.sync.dma_start(out=dt_[:, 0:nrows], in_=src, transpose=True)
                        dts.append(dt_)
                    for gc in range(8):
                        ps = ps2.tile([128, 512], F32, tag="p2")
                        for kt in range(12):
                            nc.tensor.matmul(ps[:nrows, :], dts[kt][:, 0:nrows],
                                             wdi_s[:, kt, ds(gc * 512, 512)],
                                             start=(kt == 0), stop=(kt == 11))
                        ob = wk2.tile([128, 512], BF16, tag="ob")
                        nc.scalar.activation(out=ob[:nrows, :], in_=ps[:nrows, :], func=AF.Copy)
                        nc.gpsimd.dma_start(
                            out=g2_d.rearrange("t b g -> (t b) g")[ds(nt * 128, nrows),
                                                                   ds(gc * 512, 512)],
                            in_=ob[:nrows, :])
                    # pre_mel / pre_gate from ctx k-tiles (dts[8:12])
                    psm = ps2.tile([128, N_MELS], F32, tag="pm2")
                    psg = ps2.tile([128, 1], F32, tag="pg2")
                    for kt in range(4):
                        nc.tensor.matmul(psm[:nrows, :], dts[8 + kt][:, 0:nrows],
                                         pctx_s[:, kt, :], start=(kt == 0), stop=(kt == 3))
                        nc.tensor.matmul(psg[:nrows, :], dts[8 + kt][:, 0:nrows],
                                         gctx_s[:, kt, :], start=(kt == 0), stop=(kt == 3))
                    obm = wk2.tile([128, N_MELS], F32, tag="obm")
                    obg = wk2.tile([128, 1], F32, tag="obg")
                    nc.scalar.activation(out=obm[:nrows, :], in_=psm[:nrows, :], func=AF.Copy)
                    nc.scalar.activation(out=obg[:nrows, :], in_=psg[:nrows, :], func=AF.Copy)
                    nc.gpsimd.dma_start(
                        out=premel_d.rearrange("t b g -> (t b) g")[ds(nt * 128, nrows), :],
                        in_=obm[:nrows, :])
                    nc.gpsimd.dma_start(
                        out=pregate_d.rearrange("t b g -> (t b) g")[ds(nt * 128, nrows), :],
                        in_=obg[:nrows, :])

            # ============================================ phase 3: decoder scan
            with (
                tc.tile_pool(name="c3", bufs=1) as c3,
                tc.tile_pool(name="st3", bufs=1) as st3,
                tc.tile_pool(name="wk3", bufs=1) as wk3,
                tc.tile_pool(name="ps3", bufs=4, space="PSUM") as ps3,
            ):
                wd_s = c3.tile([128, 8, GD], BF16)
                pdh_s = c3.tile([128, 8, N_MELS], BF16)
                gdh_s = c3.tile([128, 8, 1], BF16)
                ident3 = c3.tile([128, 128], F32)
                ident3_bf = c3.tile([128, 128], BF16)
                nc.gpsimd.dma_start(out=wd_s, in_=wd[:, :, :])
                nc.gpsimd.dma_start(out=pdh_s, in_=projdh[:, :, :])
                nc.gpsimd.dma_start(out=gdh_s, in_=gatedh[:, :, :])
                make_identity(nc, ident3)
                nc.vector.tensor_copy(ident3_bf, ident3)

                dhT = st3.tile([128, 8, BS], BF16)
                dc_sb = st3.tile([BS, RNN], F32)
                nc.vector.memset(dhT, 0.0)
                nc.vector.memset(dc_sb, 0.0)

                with tc.For_i(0, td) as iv:
                    g2_t = wk3.tile([BS, GD], BF16, tag="g2t")
                    nc.gpsimd.dma_start(out=g2_t, in_=g2_d[ds(iv, 1), :, :])
                    mdh_t = wk3.tile([BS, RNN], BF16, tag="mdh")
                    nc.gpsimd.dma_start(out=mdh_t, in_=mdh_i[ds(iv, 1), :, :])
                    pmel_t = wk3.tile([BS, N_MELS], F32, tag="pmel")
                    nc.gpsimd.dma_start(out=pmel_t, in_=premel_d[ds(iv, 1), :, :])
                    pgate_t = wk3.tile([BS, 1], F32, tag="pgate")
                    nc.gpsimd.dma_start(out=pgate_t, in_=pregate_d[ds(iv, 1), :, :])

                    dh_b = wk3.tile([BS, RNN], F32, tag="dhb")
                    for qd in range(2):
                        sg = {}
                        for ti, tn in enumerate("ifog"):
                            ps = ps3.tile([BS, 512], F32, tag="p3")
                            gc = qd * 4 + ti
                            nc.tensor.matmul(ps, ident3_bf[0:BS, 0:BS],
                                             g2_t[:, ds(gc * 512, 512)], start=True, stop=False)
                            for kt in range(8):
                                nc.tensor.matmul(ps, dhT[:, kt, :],
                                                 wd_s[:, kt, ds(gc * 512, 512)],
                                                 start=False, stop=(kt == 7))
                            sg[tn] = wk3.tile([BS, 512], F32, tag=f"sg{tn}", name=f"sg{tn}3")
                            nc.scalar.activation(out=sg[tn], in_=ps,
                                                 func=(AF.Tanh if tn == "g" else AF.Sigmoid))
                        qsl = ds(qd * 512, 512)
                        t1 = wk3.tile([BS, 512], F32, tag="t1")
                        t2 = wk3.tile([BS, 512], F32, tag="t2")
                        nc.vector.tensor_mul(t1, sg["f"], dc_sb[:, qsl])
                        nc.vector.tensor_mul(t2, sg["i"], sg["g"])
                        nc.vector.tensor_add(dc_sb[:, qsl], t1, t2)
                        tnc = wk3.tile([BS, 512], F32, tag="tnc")
                        nc.scalar.activation(out=tnc, in_=dc_sb[:, qsl], func=AF.Tanh)
                        nc.vector.tensor_mul(sg["o"], sg["o"], mdh_t[:, qsl])
                        nc.vector.tensor_mul(dh_b[:, qsl], sg["o"], tnc)
                    for j in range(8):
                        pst = ps3.tile([128, BS], F32, tag="p3t")
                        nc.tensor.transpose(pst, dh_b[:, ds(j * 128, 128)], ident3[0:BS, 0:BS])
                        nc.vector.tensor_copy(dhT[:, j, :], pst)
                    # mel + gate
                    psm = ps3.tile([BS, N_MELS], F32, tag="p3t")
                    nc.tensor.matmul(psm, ident3[0:BS, 0:BS], pmel_t, start=True, stop=False)
                    for j in range(8):
                        nc.tensor.matmul(psm, dhT[:, j, :], pdh_s[:, j, :],
                                         start=False, stop=(j == 7))
                    mel_sb = wk3.tile([BS, N_MELS], F32, tag="melsb")
                    nc.scalar.activation(out=mel_sb, in_=psm, func=AF.Copy)
                    nc.gpsimd.dma_start(out=mel_o[ds(iv, 1), :, :], in_=mel_sb)
                    psg = ps3.tile([BS, 1], F32, tag="p3t")
                    nc.tensor.matmul(psg, ident3[0:BS, 0:BS], pgate_t, start=True, stop=False)
                    for j in range(8):
                        nc.tensor.matmul(psg, dhT[:, j, :], gdh_s[:, j, :],
                                         start=False, stop=(j == 7))
                    gate_sb = wk3.tile([BS, 1], F32, tag="gatesb")
                    nc.scalar.activation(out=gate_sb, in_=psg, func=AF.Sigmoid)
                    nc.gpsimd.dma_start(out=gate_o[ds(iv, 1), :], in_=gate_sb)

    nc.compile()
    return nc


# ---------------------------------------------------------------- entry point

_CACHE = {}


def kernel(memory, decoder_inputs, memory_lengths, params, td=T_DEC, trace=False):
    in_maps = host_prep(memory, decoder_inputs, memory_lengths, params, td=td)
    if td not in _CACHE:
        _CACHE[td] = build_nc(td)
    nc = _CACHE[td]
    res = run_bass_kernel_spmd(nc, in_maps, core_ids=list(range(NCORES)), trace=trace)
    mels, gates, aligns = [], [], []
    for c in range(NCORES):
        r = res.results[c]
        mels.append(r["mel_o"])
        gates.append(r["gate_o"])
        aligns.append(r["align_o"])
    mel = np.concatenate(mels, axis=1)      # (td,B,80)
    gate = np.concatenate(gates, axis=1)    # (td,B)
    align = np.concatenate(aligns, axis=1)  # (td,B,TE)
    out = (np.ascontiguousarray(np.transpose(mel, (1, 2, 0))),
           np.ascontiguousarray(np.transpose(gate, (1, 0))),
           np.ascontiguousarray(np.transpose(align, (1, 0, 2))))
    kernel._last_result = res
    return out
